# revision 27
# baseline (speedup 1.0000x reference)
"""Causal self-attention Trainium2 kernel v2 (B=4, T=2048, C=1024, H=16, D=64).

Sharding: 8 cores = 4 batches x 2 head-groups. Core c handles batch c//2 and
heads 8*(c%2) .. 8*(c%2)+8 (as 4 pairs of 2 heads). Each core computes its
QKV column slice, causal attention for its 8 heads, and a partial projection
(w_proj row slice); the host sums the two partials per batch. No collectives.

v2 changes vs v1:
- bf16 everywhere on the matmul path (inputs shipped as bf16; PSUM stays f32).
- No gpsimd at all (no library load, no affine_select, no partition_broadcast):
  causal masking of diagonal blocks = DVE multiply with 4 static 0/1 mask
  patterns shipped via an aux DRAM input; the identity for the PE transpose
  comes from the same aux tensor.
- Softmax row-sums: V_ext = [V | 1...1] with 64 ones columns, so the PV matmul
  replicates the row-sum into partitions 64:128 of the O tile; normalization is
  a straight [64,512] reciprocal + multiply (no partition broadcast).
- Attention groups are one 128-key block (both heads in one [128,2,512] PSUM
  tile): a single exp per group covers both heads.
- QKV of pair p+1 (and for the last pair, the output projection) is emitted as
  filler work interleaved into attention's group loop so the PE never idles
  while ACT paces the softmax.
"""

import os
import sys

for _p in ("/opt/trn_rl_repo", "/root/.axon_site/_ro/trn_rl_repo"):
    if os.path.isdir(_p) and _p not in sys.path:
        sys.path.insert(0, _p)

import numpy as np
import ml_dtypes

import concourse.bacc as bacc
import concourse.bass as bass
import concourse.mybir as mybir
import concourse.tile as tile
from concourse import bass_utils
from concourse.bass import ds, ts

F32 = mybir.dt.float32
BF16 = mybir.dt.bfloat16
BF16NP = ml_dtypes.bfloat16

B, T, C, H, D = 4, 2048, 1024, 16, 64
NCORES = 8
HPC = 8          # heads per core
NPAIR = 4        # head pairs per core
SCALE = 1.0 / 8.0  # 1/sqrt(D)
CT = C // 128    # 8 C-tiles
NKB = T // 128   # 16 key blocks
NMACRO = 4       # query macro tiles of 512
AUXW = 4 * 2 * 512 + 128  # 4 masks (dup per head) + identity

_CACHE = {}


def _build(finalize=True, reps=1):
    nc = bacc.Bacc(None, target_bir_lowering=False)

    xt = nc.dram_tensor("xt", [C, T], BF16, kind="ExternalInput")
    wqkv = nc.dram_tensor("wqkv", [NPAIR, 128, 3, CT, 128], BF16, kind="ExternalInput")
    wp = nc.dram_tensor("wp", [512, C], BF16, kind="ExternalInput")
    aux = nc.dram_tensor("aux", [128, AUXW], BF16, kind="ExternalInput")
    out = nc.dram_tensor("out", [T, C], F32, kind="ExternalOutput")

    xt_r = xt.rearrange("(o p) t -> p o t", p=128)       # [128, 8, 2048]
    wp_r = wp.rearrange("(o p) n -> p o n", p=128)       # [128, 4, 1024]

    with tile.TileContext(nc) as tc:
        with (
            tc.tile_pool(name="fixed", bufs=1) as fixed,
            tc.tile_pool(name="wpool", bufs=2) as wpool,
            tc.tile_pool(name="qt", bufs=2) as qt_pool,
            tc.tile_pool(name="kt", bufs=2) as kt_pool,
            tc.tile_pool(name="vt", bufs=2) as vt_pool,
            tc.tile_pool(name="vext", bufs=2) as vext_pool,
            tc.tile_pool(name="pt", bufs=8) as pt_pool,
            tc.tile_pool(name="small", bufs=8) as small,
            tc.tile_pool(name="ost", bufs=3) as ost_pool,
            tc.tile_pool(name="psum_s", bufs=2, space="PSUM") as psum_s,
            tc.tile_pool(name="psum_qkv", bufs=2, space="PSUM") as psum_qkv,
            tc.tile_pool(name="psum_o", bufs=2, space="PSUM") as psum_o,
        ):
            # --- persistent tiles ---
            xt_sb = [fixed.tile([128, T], BF16, name=f"xt{o}") for o in range(CT)]
            aux_sb = fixed.tile([128, AUXW], BF16, name="aux")
            masks = aux_sb[:, 0 : 4 * 2 * 512].rearrange(
                "p (d h q) -> p d h q", d=4, h=2
            )
            ident = aux_sb[:, 4 * 2 * 512 :]
            yt = fixed.tile([128, NPAIR, T], BF16, name="yt")  # y^T: head-dim on partitions

            aux_loaded = [False]
            mask_dma = [None]

            for rep in range(reps):
                pass

                def s_tile():
                    return psum_s.tile([128, 2, 512], F32, tag="s", name="s_ps")

                def o_tile():
                    return psum_o.tile([128, 512], F32, tag="o", name="o_ps")

                def qkv_tile():
                    return psum_qkv.tile([128, 512], F32, tag="qkv", name="q_ps")

                ones_src = xt_sb[0][:, 0:1024].rearrange("p (b a) -> p b a", a=64)

                def emit_w_dma(p, w_sb, j):
                    nc.sync.dma_start(w_sb[:, j], wqkv[p, :, j])

                def emit_qkv_half(w_sb, dst, j, tchunk, half, ps_box):
                    # half 0: k=0..3 into fresh psum; half 1: k=4..7 + copy out
                    if half == 0:
                        ps_box[0] = qkv_tile()
                    ps = ps_box[0]
                    for k in range(4 * half, 4 * half + 4):
                        nc.tensor.matmul(
                            ps[:],
                            w_sb[:, j, k, :],
                            xt_sb[k][:, ts(tchunk, 512)],
                            start=(k == 0),
                            stop=(k == CT - 1),
                            skip_group_check=True,
                        )
                    if half == 1:
                        nc.vector.tensor_copy(out=dst[:, ts(tchunk, 512)], in_=ps[:])

                def emit_ones(ve):
                    for lo in (64, 192):
                        nc.vector.tensor_scalar(
                            ve[:, :, lo : lo + 64],
                            ones_src,
                            0.0,
                            1.0,
                            mybir.AluOpType.mult,
                            mybir.AluOpType.add,
                        )

                def emit_transpose(vt, ve, kb):
                    tp = psum_qkv.tile([128, 128], BF16, tag="qkv", name="tp_ps")
                    nc.tensor.transpose(tp[:, 0:128], vt[:, ts(kb, 128)], ident)
                    # single strided copy: tp cols (h*64+d) -> ve[:, kb, 128h+d]
                    nc.vector.tensor_copy(
                        out=ve[:, kb].rearrange("p (h c) -> p h c", h=2)[:, :, 0:64],
                        in_=tp[:, 0:128].rearrange("p (h c) -> p h c", h=2),
                    )

                def make_pair_tiles():
                    return dict(
                        qt=qt_pool.tile([128, T], BF16, tag="qt", name="qt_t"),
                        kt=kt_pool.tile([128, T], BF16, tag="kt", name="kt_t"),
                        vt=vt_pool.tile([128, T], BF16, tag="vt", name="vt_t"),
                        ve=vext_pool.tile([128, NKB, 256], BF16, tag="ve", name="ve_t"),
                    )

                def pair_filler_units(p, tiles):
                    """Closures producing qt/kt/vt/ve for pair p, in order.
                    Returns (dma_units, compute_units)."""
                    dma_units = []
                    units = []
                    w_sb = wpool.tile([128, 3, CT, 128], BF16, tag="w", name="w_sb")
                    for j in range(3):
                        dma_units.append(lambda j=j: emit_w_dma(p, w_sb, j))
                    if not aux_loaded[0]:
                        aux_loaded[0] = True
                        # identity (32KB) now; the 1MB masks after the xt stream
                        dma_units.insert(
                            0,
                            lambda: nc.sync.dma_start(
                                aux_sb[:, 4 * 2 * 512 :], aux[:, 4 * 2 * 512 :]
                            ),
                        )
                        mask_dma[0] = lambda: nc.sync.dma_start(
                            aux_sb[:, 0 : 4 * 2 * 512], aux[:, 0 : 4 * 2 * 512]
                        )
                    units.append(lambda: emit_ones(tiles["ve"]))
                    for tchunk in range(4):
                        for dst, j in (
                            (tiles["qt"], 0),
                            (tiles["kt"], 1),
                            (tiles["vt"], 2),
                        ):
                            box = [None]
                            for half in range(2):
                                units.append(
                                    lambda dst=dst, j=j, tc_=tchunk, h=half, b=box, w=w_sb: emit_qkv_half(
                                        w, dst, j, tc_, h, b
                                    )
                                )
                        for kb in range(4 * tchunk, 4 * tchunk + 4):
                            units.append(
                                lambda kb=kb: emit_transpose(
                                    tiles["vt"], tiles["ve"], kb
                                )
                            )
                    return dma_units, units

                # --- wp prefetch + projection (used as filler for last pair) ---
                wp01 = wpool.tile([128, 2, 1024], BF16, tag="w", name="wp01")
                wp23 = wpool.tile([128, 2, 1024], BF16, tag="w", name="wp23")

                def emit_wp_dma():
                    nc.sync.dma_start(wp01[:], wp_r[:, 0:2, :])
                    nc.sync.dma_start(wp23[:], wp_r[:, 2:4, :])

                def emit_proj(tt, split_dma=False):
                    pss = []
                    for oc in range(2):
                        ps = qkv_tile()
                        for p in range(NPAIR):
                            wsrc = wp01 if p < 2 else wp23
                            nc.tensor.matmul(
                                ps[:],
                                yt[:, p, ts(tt, 128)],
                                wsrc[:, p % 2, ts(oc, 512)],
                                start=(p == 0),
                                stop=(p == NPAIR - 1),
                                skip_group_check=True,
                            )
                        pss.append(ps)
                    ost = ost_pool.tile([128, 2, 512], F32, tag="ost", name="ost_t")
                    if split_dma:
                        # tail-only: per-half DMA so the last writes overlap
                        for oc in range(2):
                            nc.vector.tensor_copy(out=ost[:, oc, :], in_=pss[oc][:])
                            nc.sync.dma_start(
                                out[ts(tt, 128), ts(oc, 512)], ost[:, oc, :]
                            )
                    else:
                        for oc in range(2):
                            nc.vector.tensor_copy(out=ost[:, oc, :], in_=pss[oc][:])
                        nc.sync.dma_start(
                            out[ts(tt, 128), :], ost.rearrange("p a b -> p (a b)")
                        )

                # --- attention for one pair, consuming filler units ---
                def attention(p, tiles, filler, proj_after_macro=None):
                    qt, kt, ve = tiles["qt"], tiles["kt"], tiles["ve"]

                    def emit_filler():
                        if filler:
                            u = filler.pop(0)
                            if u is not None:
                                u()

                    for i in range(NMACRO):
                        nblk = 4 * (i + 1)
                        o_ps = (o_tile(), o_tile())

                        def do_pv(kb, pt, off, i=i, nblk=nblk, o_ps=o_ps):
                            for h in range(2):
                                nc.tensor.matmul(
                                    o_ps[h][:, off:512],
                                    ve[:, kb, ds(128 * h, 128)],
                                    pt[:, h, off:512],
                                    start=(kb == 0),
                                    stop=(kb == nblk - 1),
                                    skip_group_check=True,
                                )

                        pending = []
                        for kb in range(nblk):
                            d = kb - 4 * i
                            off = 128 * d if d >= 0 else 0
                            st_t = s_tile()
                            for h in range(2):
                                nc.tensor.matmul(
                                    st_t[:, h, off:512],
                                    kt[ds(64 * h, 64), ts(kb, 128)],
                                    qt[ds(64 * h, 64), ds(512 * i + off, 512 - off)],
                                    tile_position=(64 * h, 0),
                                )
                            emit_filler()
                            pt = pt_pool.tile([128, 2, 512], BF16, tag="pt", name="pt_t")
                            nc.scalar.activation(
                                out=pt[:, :, off:512],
                                in_=st_t[:, :, off:512],
                                func=mybir.ActivationFunctionType.Exp,
                                bias=0.0,
                                scale=SCALE,
                            )
                            if d >= 0:
                                nc.vector.tensor_tensor(
                                    pt[:, :, off:512],
                                    pt[:, :, off:512],
                                    masks[:, d, :, off:512],
                                    mybir.AluOpType.mult,
                                )
                            pending.append((kb, pt, off))
                            if len(pending) > 1:
                                do_pv(*pending.pop(0))
                        do_pv(*pending.pop(0))

                        # normalize: partitions 64:128 of o_ps hold the row-sums
                        for h in range(2):
                            recip = small.tile([64, 512], F32, tag="recip", name="recip_t")
                            nc.vector.reciprocal(recip[:], o_ps[h][64:128, :])
                            nc.vector.tensor_tensor(
                                yt[ds(64 * h, 64), p, ts(i, 512)],
                                o_ps[h][0:64, :],
                                recip[:],
                                mybir.AluOpType.mult,
                            )
                        if proj_after_macro is not None and i > 0:
                            proj_after_macro(i - 1)

                    # drain leftover filler
                    while filler:
                        u = filler.pop(0)
                        if u is not None:
                            u()

                # --- prologue: pair 0 w DMAs, then xt (tchunk-major), then QKV ---
                tiles0 = make_pair_tiles()
                dma0, units0 = pair_filler_units(0, tiles0)
                for u in dma0[:2]:
                    u()  # identity + q-weights first
                for o in range(CT):
                    nc.sync.dma_start(
                        xt_sb[o][:, ts(0, 512)], xt_r[:, o, ts(0, 512)]
                    )
                for u in dma0[2:]:
                    u()
                if mask_dma[0] is not None:
                    mask_dma[0]()
                    mask_dma[0] = None
                for tchunk in range(1, 4):
                    for o in range(CT):
                        nc.sync.dma_start(
                            xt_sb[o][:, ts(tchunk, 512)],
                            xt_r[:, o, ts(tchunk, 512)],
                        )
                # emit only the tc0 slice of pair-0 QKV up front; the rest
                # streams into attention(0) as filler alongside pair 1's units
                for u in units0[:11]:
                    u()
                rest0 = units0[11:]

                # --- pairs 0..2: attention(p) with QKV(p+1) as filler ---
                cur = tiles0
                for p in range(3):
                    nxt = make_pair_tiles()
                    dman, unitsn = pair_filler_units(p + 1, nxt)
                    attention(p, cur, rest0 + dman + unitsn)
                    rest0 = []
                    cur = nxt

                # --- pair 3: attention with wp prefetch + projection filler ---
                emit_wp_dma()
                proj_done = [False] * NMACRO
                filler3 = []

                def proj_after_macro(i):
                    if not proj_done[i]:
                        proj_done[i] = True
                        for tt in range(4 * i, 4 * i + 4):
                            filler3.append(
                                lambda tt=tt: emit_proj(tt, split_dma=(tt >= 14))
                            )

                attention(3, cur, filler3, proj_after_macro=proj_after_macro)
                for i in range(NMACRO):
                    proj_after_macro(i)
                while filler3:
                    u = filler3.pop(0)
                    if u is not None:
                        u()

    if finalize:
        nc.finalize()
    return nc


def _aux_host():
    p = np.arange(128)[:, None]
    q = np.arange(512)[None, :]
    masks = np.zeros((128, 4, 2, 512), dtype=BF16NP)
    for d in range(4):
        m = (q >= p + 128 * d).astype(np.float32).astype(BF16NP)
        masks[:, d, 0, :] = m
        masks[:, d, 1, :] = m
    ident = np.eye(128, dtype=np.float32).astype(BF16NP)
    return np.concatenate([masks.reshape(128, 4 * 2 * 512), ident], axis=1)


def _shard_inputs(x, w_qkv, w_proj):
    """Build the per-core input maps (host-side data marshalling only)."""
    aux = _aux_host()
    in_maps = []
    for c in range(NCORES):
        b, g = c // 2, c % 2
        xt = np.ascontiguousarray(x[b].T).astype(BF16NP)  # [C, T]
        wqkv = np.empty((NPAIR, 128, 3, CT, 128), dtype=BF16NP)
        wq4 = w_qkv.reshape(CT, 128, 3, C)  # (o, q, j, cols-within-j)
        for p in range(NPAIR):
            col = 64 * (8 * g + 2 * p)
            blk = wq4[:, :, :, col : col + 128]  # (o, q, j, m)
            wqkv[p] = blk.transpose(1, 2, 0, 3).astype(BF16NP)
        wp = np.ascontiguousarray(w_proj[512 * g : 512 * g + 512, :]).astype(BF16NP)
        in_maps.append({"xt": xt, "wqkv": wqkv, "wp": wp, "aux": aux})
    return in_maps


LAST_RESULT = None  # BassKernelResults of the most recent run (for profiling)


def _build_baseline():
    """Same external I/O as the real kernel, trivial body — used to measure
    and subtract the per-dispatch transport overhead of the runtime."""
    nc = bacc.Bacc(None, target_bir_lowering=False)
    nc.dram_tensor("xt", [C, T], BF16, kind="ExternalInput")
    nc.dram_tensor("wqkv", [NPAIR, 128, 3, CT, 128], BF16, kind="ExternalInput")
    wp = nc.dram_tensor("wp", [512, C], BF16, kind="ExternalInput")
    nc.dram_tensor("aux", [128, AUXW], BF16, kind="ExternalInput")
    out = nc.dram_tensor("out", [T, C], F32, kind="ExternalOutput")
    with tile.TileContext(nc) as tc:
        with tc.tile_pool(name="p", bufs=1) as pool:
            t = pool.tile([128, 128], F32)
            nc.sync.dma_start(t[:], wp[0:128, 0:256].bitcast(F32))
            for tt in range(T // 128):
                nc.sync.dma_start(
                    out.rearrange("(a p) c -> p a c", p=128)[:, tt, 0:128], t[:]
                )
    nc.finalize()
    return nc


def time_kernel(x, w_qkv, w_proj, iters=32):
    """Estimate per-invocation HW time by comparing against a trivial NEFF
    with identical I/O (amortizes the runtime's dispatch overhead)."""
    import time as _time

    import jax
    from jax.sharding import Mesh, PartitionSpec
    from jax.experimental.shard_map import shard_map

    import concourse.mybir as _mybir
    from concourse import bass2jax as _b2j

    if "nc" not in _CACHE:
        _CACHE["nc"] = _build()
    _b2j.install_neuronx_cc_hook()

    in_maps = _shard_inputs(x, w_qkv, w_proj)
    devices = jax.devices()[:NCORES]
    mesh = Mesh(np.asarray(devices), ("core",))

    def measure(nc):
        part_name = nc.partition_id_tensor.name if nc.partition_id_tensor else None
        in_names, out_names, out_avals = [], [], []
        for alloc in nc.m.functions[0].allocations:
            if not isinstance(alloc, _mybir.MemoryLocationSet):
                continue
            name = alloc.memorylocations[0].name
            if alloc.kind == "ExternalInput":
                if name != part_name:
                    in_names.append(name)
            elif alloc.kind == "ExternalOutput":
                out_names.append(name)
                out_avals.append(
                    jax.core.ShapedArray(
                        tuple(alloc.tensor_shape), _mybir.dt.np(alloc.dtype)
                    )
                )
        n_params = len(in_names)
        all_names = tuple(in_names + out_names + ([part_name] if part_name else []))

        def _b(*args):
            operands = list(args)
            if part_name:
                operands.append(_b2j.partition_id_tensor())
            return tuple(
                _b2j._bass_exec_p.bind(
                    *operands,
                    out_avals=tuple(out_avals),
                    in_names=all_names,
                    out_names=tuple(out_names),
                    lowering_input_output_aliases=(),
                    sim_require_finite=True,
                    sim_require_nnan=True,
                    nc=nc,
                )
            )

        per_core = [[np.asarray(m[nm]) for nm in in_names] for m in in_maps]
        concat_in = [
            np.concatenate([per_core[c][i] for c in range(NCORES)], axis=0)
            for i in range(n_params)
        ]
        concat_zeros = [
            np.zeros((NCORES * av.shape[0], *av.shape[1:]), av.dtype)
            for av in out_avals
        ]
        nio = n_params + len(out_names)
        sharding = jax.sharding.NamedSharding(mesh, PartitionSpec("core"))
        dev_args = [jax.device_put(a, sharding) for a in (*concat_in, *concat_zeros)]
        fn = jax.jit(
            shard_map(
                _b,
                mesh=mesh,
                in_specs=(PartitionSpec("core"),) * nio,
                out_specs=(PartitionSpec("core"),) * len(out_names),
                check_rep=False,
            )
        )
        jax.block_until_ready(fn(*dev_args))  # compile + warmup

        def run():
            t0 = _time.perf_counter()
            jax.block_until_ready(fn(*dev_args))
            return _time.perf_counter() - t0

        return run

    # Alternate the two NEFFs so slow wall-clock drift cancels; the first
    # call after each NEFF switch pays executable-swap overhead, so time
    # the second call of each pair.
    runk = measure(_CACHE["nc"])
    runb = measure(_build_baseline())
    sk, sb = [], []
    for _ in range(iters):
        runk()
        sk.append(runk())
        runb()
        sb.append(runb())
    sk.sort()
    sb.sort()
    timings = {
        "kernel_min_s": sk[0],
        "baseline_min_s": sb[0],
        "kernel_median_s": sk[len(sk) // 2],
        "baseline_median_s": sb[len(sb) // 2],
    }
    per_iter_ns = max(sk[0] - sb[0], 0.0) * 1e9
    return per_iter_ns, timings


def kernel(x, w_qkv, w_proj, trace=False):
    global LAST_RESULT
    x = np.asarray(x, dtype=np.float32)
    w_qkv = np.asarray(w_qkv, dtype=np.float32)
    w_proj = np.asarray(w_proj, dtype=np.float32)

    if "nc" not in _CACHE:
        _CACHE["nc"] = _build()
    nc = _CACHE["nc"]

    in_maps = _shard_inputs(x, w_qkv, w_proj)
    try:
        res = bass_utils.run_bass_kernel_spmd(
            nc, in_maps, core_ids=list(range(NCORES)), trace=trace
        )
    except Exception:
        if not trace:
            raise
        # tracing unavailable in this environment (no NTFF hook) — run plain
        res = bass_utils.run_bass_kernel_spmd(
            nc, in_maps, core_ids=list(range(NCORES)), trace=False
        )
    LAST_RESULT = res

    out = np.empty((B, T, C), dtype=np.float32)
    for b in range(B):
        out[b] = res.results[2 * b]["out"] + res.results[2 * b + 1]["out"]
    return out
